# revision 13
# baseline (speedup 1.0000x reference)
"""Trainium2 Bass kernel for nn_MemoryGraph (gnn_message_passing).

Sharding: N_cells=64 split 8 ways -> 8 cells per NeuronCore. Each cell is
fully independent (per-cell weights + per-cell state), so there are no
collectives. Batch B=8 stays whole on every core.

Device-side layout is feature-major ([feat, token]) everywhere so that every
matmul contracts over the partition dim with zero on-device transposes; all
transposes are host-side numpy work:
  in : h^T, W^T, neuron_id^T, inject_w^T, H_aug^T, decay_logit (pair-packed)
  out: h_new^T, msg_new^T, (W+delta_W)^T, decay-delta pair-packed
The modulator's big matmul streams mod_w2 for a *pair* of cells as a
[128, 16512] stationary block (two 64-row hh blocks stacked -- contiguous in
DRAM) against a zero-padded [128, 16] hidden block, giving delta^T directly
partition-dense in PSUM. mod_b2 is added on host afterwards.
"""

import sys

for _p in ("/opt/trn_rl_repo",):
    if _p not in sys.path:
        sys.path.insert(0, _p)

import numpy as np

import concourse.bass as bass
import concourse.tile as tile
from concourse import bacc, mybir
from concourse.bass_utils import run_bass_kernel_spmd

F32 = mybir.dt.float32
BF16 = mybir.dt.bfloat16
F32R = mybir.dt.float32r
AX = mybir.AxisListType
AF = mybir.ActivationFunctionType

B, NC, N, DN, ALPHA = 8, 64, 128, 64, 2
HS, HM, HMOD = 256, 256, 64
MOD_IN, MOD_OUT = 2 * DN + 5, N * N + N
NCORES = 8
NCL = NC // NCORES          # cells per core = 8
NPAIR = NCL // 2            # cell pairs per core = 4
W2C = 4096                  # mod_w2 free-dim chunk (32 i-blocks)
NCHUNK = (N * N) // W2C     # 4 chunks for the delta_W part
IBLK = W2C // N             # i-blocks per chunk = 32


def build_bass():
    nc = bacc.Bacc("TRN2", target_bir_lowering=False, debug=False)

    def din(name, shape):
        return nc.dram_tensor(name, list(shape), F32, kind="ExternalInput").ap()

    def dout(name, shape):
        return nc.dram_tensor(name, list(shape), F32, kind="ExternalOutput").ap()

    hT = din("hT", (NCL, DN, B, N))            # h^T packed per cell
    msgN = din("msgN", (NCL, N, B, DN))        # msg [j, (b, d)] packed per cell
    WT = din("WT", (NCL, N, B, N))             # W^T packed per cell: [j, (b, i)]
    w2s = din("w2s", (NPAIR, 2 * HMOD, MOD_OUT))  # stacked mod_w2 per pair
    nidT = din("nidT", (NCL, DN, N))
    iwT = din("iwT", (NCL, DN, ALPHA * DN))
    injbT = din("injbT", (ALPHA * DN, NCL))
    haT = din("haT", (NCL, DN, B))
    sc5 = din("sc5", (NCL, 5, B))
    m1w = din("m1w", (NCL, MOD_IN, HMOD))      # mod_w1 (rows 0:128 prescaled by 1/N)
    m1bT = din("m1bT", (HMOD, NCL))
    sw1T = din("sw1T", (3 * DN, HS))
    sb1T2 = din("sb1T2", (128, 2))
    sw2T = din("sw2T", (HS, DN))
    sb2T = din("sb2T", (DN, 1))
    mw1T = din("mw1T", (2 * DN, HM))
    mb1T2 = din("mb1T2", (128, 2))
    mw2T = din("mw2T", (HM, DN))
    mb2T = din("mb2T", (DN, 1))
    dlT = din("dlT", (NPAIR, N, 16))           # decay_logit [pair, n, (cell,b)]

    hnT = dout("hnT", (NCL, DN, B, N))
    mnT = dout("mnT", (NCL, DN, B, N))
    WnT = dout("WnT", (NCL, N, B, N))          # (W + delta_W)^T, [j, (b, i)]
    dnT = dout("dnT", (NPAIR, N, 16))          # decay_logit + delta_decay

    def r(ap):
        return ap.bitcast(F32R)

    with tile.TileContext(nc) as tc:
        with (
            tc.tile_pool(name="const", bufs=1) as constp,
            tc.tile_pool(name="cellc", bufs=2) as cellp,
            tc.tile_pool(name="pairc", bufs=2) as pairp,
            tc.tile_pool(name="wt", bufs=4) as wtp,
            tc.tile_pool(name="wn", bufs=4) as wnp,
            tc.tile_pool(name="w2", bufs=2) as w2p,
            tc.tile_pool(name="acts", bufs=4) as actp,
            tc.tile_pool(name="outs", bufs=3) as outp,
            tc.tile_pool(name="ps_l1", bufs=2, space="PSUM") as psl1,
            tc.tile_pool(name="ps_s", bufs=3, space="PSUM") as pss,
            tc.tile_pool(name="ps_d", bufs=2, space="PSUM") as psd,
        ):
            # ---- resident shared weights (round-copied to f32r for PE) ----
            stg = constp.tile([128, 2 * HS], F32, tag="stg")
            sw1a = constp.tile([128, HS], F32, tag="sw1a")   # state_w1^T rows 0:128
            sw1b = constp.tile([64, HS], F32, tag="sw1b")    # rows 128:192 (nid part)
            nc.sync.dma_start(out=stg[:, 0:HS], in_=sw1T[0:128, :])
            nc.sync.dma_start(out=stg[0:64, HS:2 * HS], in_=sw1T[128:192, :])
            nc.vector.tensor_copy(sw1a.bitcast(F32R), stg[:, 0:HS])
            nc.vector.tensor_copy(sw1b.bitcast(F32R), stg[0:64, HS:2 * HS])
            sb1 = constp.tile([128, 2], F32, tag="sb1")
            nc.sync.dma_start(out=sb1, in_=sb1T2)
            stg2 = constp.tile([128, 2, DN], F32, tag="stg2")
            sw2k = constp.tile([128, 2, DN], F32, tag="sw2k")  # state_w2^T k-chunks
            nc.sync.dma_start(out=stg2[:, 0, :], in_=sw2T[0:128, :])
            nc.sync.dma_start(out=stg2[:, 1, :], in_=sw2T[128:256, :])
            nc.vector.tensor_copy(sw2k.bitcast(F32R), stg2)
            sb2 = constp.tile([DN, 1], F32, tag="sb2")
            nc.sync.dma_start(out=sb2, in_=sb2T)
            stg3 = constp.tile([128, HM], F32, tag="stg3")
            mw1 = constp.tile([128, HM], F32, tag="mw1")
            nc.sync.dma_start(out=stg3, in_=mw1T)
            nc.vector.tensor_copy(mw1.bitcast(F32R), stg3)
            mb1 = constp.tile([128, 2], F32, tag="mb1")
            nc.sync.dma_start(out=mb1, in_=mb1T2)
            stg4 = constp.tile([128, 2, DN], F32, tag="stg4")
            mw2k = constp.tile([128, 2, DN], F32, tag="mw2k")
            nc.sync.dma_start(out=stg4[:, 0, :], in_=mw2T[0:128, :])
            nc.sync.dma_start(out=stg4[:, 1, :], in_=mw2T[128:256, :])
            nc.vector.tensor_copy(mw2k.bitcast(F32R), stg4)
            mb2 = constp.tile([DN, 1], F32, tag="mb2")
            nc.sync.dma_start(out=mb2, in_=mb2T)
            injb = constp.tile([ALPHA * DN, NCL], F32, tag="injb")
            nc.sync.dma_start(out=injb, in_=injbT)
            m1b = constp.tile([HMOD, NCL], F32, tag="m1b")
            nc.sync.dma_start(out=m1b, in_=m1bT)

            # ~4us of dummy matmuls to take the PE HAM clock-gate to 8/8
            # before the first real matmuls arrive.
            ps_warm = pss.tile([128, HS], F32, tag="ps_warm", bufs=1)
            for _ in range(40):
                nc.tensor.matmul(ps_warm, r(sw1a[:, 0:128]), r(sw1a),
                                 start=True, stop=True)

            for p in range(NPAIR):
                hid2 = pairp.tile([128, 16], BF16, tag="hid2")
                nc.vector.memset(hid2, 0.0)
                wts = [None, None]
                wns = [None, None]

                for cell in range(2):
                    c = 2 * p + cell
                    nid_c = cellp.tile([DN, N], F32, tag="nid")
                    nc.sync.dma_start(out=nid_c, in_=nidT[c])
                    nid4 = cellp.tile([DN, 4, N], F32, tag="nid4")
                    for bl in range(4):
                        nc.vector.tensor_copy(nid4[:, bl, :].bitcast(F32R), nid_c)
                    iw_c = cellp.tile([DN, ALPHA * DN], F32, tag="iw")
                    nc.sync.dma_start(out=iw_c, in_=iwT[c])
                    ha_c = cellp.tile([DN, B], F32, tag="ha")
                    nc.sync.dma_start(out=ha_c, in_=haT[c])
                    m1wa = cellp.tile([128, HMOD], F32, tag="m1wa")
                    nc.sync.dma_start(out=m1wa, in_=m1w[c, 0:128, :])
                    m1wb = cellp.tile([5, HMOD], F32, tag="m1wb")
                    nc.sync.dma_start(out=m1wb, in_=m1w[c, 128:133, :])
                    mi_a = cellp.tile([128, B], F32, tag="mi_a")
                    mi_b = cellp.tile([5, B], F32, tag="mi_b")
                    nc.sync.dma_start(out=mi_b, in_=sc5[c])

                    # inject = (H_aug cell slice) @ inject_w^T, per ALPHA row
                    pi = pss.tile([DN, ALPHA, B], F32, tag="ps_small")
                    for a in range(ALPHA):
                        nc.tensor.matmul(
                            pi[:, a, :], iw_c[:, a * DN:(a + 1) * DN], ha_c,
                            start=True, stop=True,
                        )
                    inj = cellp.tile([DN, ALPHA, B], F32, tag="inj")
                    for a in range(ALPHA):
                        nc.scalar.activation(
                            inj[:, a, :], pi[:, a, :], AF.Identity,
                            bias=injb[a * DN:(a + 1) * DN, c:c + 1],
                        )

                    # per-cell bulk loads (big DMA descriptors)
                    wt_c = wtp.tile([N, B, N], F32, tag="wt",
                                    name=f"wt_{p}_{cell}")
                    wts[cell] = wt_c
                    nc.sync.dma_start(out=wt_c, in_=WT[c])
                    msg_c = cellp.tile([N, B, DN], F32, tag="msgc")
                    nc.sync.dma_start(out=msg_c, in_=msgN[c])

                    st8 = actp.tile([128, B, N], F32, tag="st")
                    mg8 = actp.tile([128, B, N], F32, tag="mg")
                    mn8 = outp.tile([DN, B, N], F32, tag="mn")
                    hstg = cellp.tile([DN, B, N], F32, tag="hstg")
                    nc.sync.dma_start(out=hstg, in_=hT[c])
                    nc.scalar.copy(st8[0:DN].bitcast(F32R), hstg)

                    for b in range(B):
                        # received^T = (msg^T) @ (W^T) : [d, i]
                        pr = pss.tile([DN, N], F32, tag="ps_small")
                        nc.tensor.matmul(pr, msg_c[:, b, :], wt_c[:, b, :],
                                         start=True, stop=True)
                        nc.vector.tensor_add(
                            pr[:, 0:ALPHA], pr[:, 0:ALPHA], inj[:, :, b]
                        )
                        nc.vector.tensor_copy(st8[DN:128, b, :].bitcast(F32R),
                                              pr)
                    for g in range(2):
                        nc.vector.tensor_copy(
                            mg8[DN:128, 4 * g:4 * g + 4, :].bitcast(F32R),
                            st8[DN:128, 4 * g:4 * g + 4, :])

                    for g in range(2):          # token batches of 4*N
                        g4 = slice(4 * g, 4 * g + 4)
                        # state MLP layer 1
                        hid1 = actp.tile([128, 2, 4 * N], F32, tag="hid1")
                        for m in range(2):
                            ps1 = psl1.tile([128, 4 * N], F32, tag="ps_l1")
                            nc.tensor.matmul(
                                ps1, r(sw1a[:, m * 128:(m + 1) * 128]),
                                r(st8[:, g4, :]), start=True, stop=False,
                            )
                            nc.tensor.matmul(
                                ps1, r(sw1b[:, m * 128:(m + 1) * 128]), r(nid4),
                                start=False, stop=True,
                            )
                            nc.scalar.activation(
                                hid1[:, m, :].bitcast(F32R), ps1, AF.Tanh,
                                bias=sb1[:, m:m + 1]
                            )
                        # state MLP layer 2 -> h_new^T into mg8 rows 0:64
                        ps2 = pss.tile([DN, 4 * N], F32, tag="ps_small")
                        nc.tensor.matmul(ps2, r(sw2k[:, 0, :]),
                                         r(hid1[:, 0, :]), start=True, stop=False)
                        nc.tensor.matmul(ps2, r(sw2k[:, 1, :]),
                                         r(hid1[:, 1, :]), start=False, stop=True)
                        mg8f = mg8[0:DN, g4, :].rearrange("d b n -> d (b n)")
                        nc.scalar.activation(mg8f.bitcast(F32R), ps2, AF.Tanh,
                                             bias=sb2)
                        for bl in range(4):
                            b = g * 4 + bl
                            nc.vector.reduce_sum(mi_a[0:DN, b:b + 1],
                                                 mg8[0:DN, b, :], axis=AX.X)

                        # msg MLP
                        mid1 = actp.tile([128, 2, 4 * N], F32, tag="mid1")
                        for m in range(2):
                            ps1 = psl1.tile([128, 4 * N], F32, tag="ps_l1")
                            nc.tensor.matmul(
                                ps1, r(mw1[:, m * 128:(m + 1) * 128]),
                                r(mg8[:, g4, :]), start=True, stop=True,
                            )
                            nc.scalar.activation(
                                mid1[:, m, :].bitcast(F32R), ps1, AF.Tanh,
                                bias=mb1[:, m:m + 1]
                            )
                        ps3 = pss.tile([DN, 4 * N], F32, tag="ps_small")
                        nc.tensor.matmul(ps3, r(mw2k[:, 0, :]),
                                         r(mid1[:, 0, :]), start=True, stop=False)
                        nc.tensor.matmul(ps3, r(mw2k[:, 1, :]),
                                         r(mid1[:, 1, :]), start=False, stop=True)
                        mn8f = mn8[:, g4, :].rearrange("d b n -> d (b n)")
                        nc.scalar.activation(mn8f, ps3, AF.Tanh, bias=mb2)
                        for bl in range(4):
                            b = g * 4 + bl
                            nc.vector.reduce_sum(mi_a[DN:128, b:b + 1],
                                                 mn8[:, b, :], axis=AX.X)

                    nc.sync.dma_start(out=hnT[c], in_=mg8[0:DN])
                    nc.sync.dma_start(out=mnT[c], in_=mn8)

                    # modulator hidden for this cell -> hid2 block (bf16)
                    pm = pss.tile([HMOD, B], F32, tag="ps_small")
                    nc.tensor.matmul(pm, m1wa, mi_a, start=True, stop=False)
                    nc.tensor.matmul(pm, m1wb, mi_b, start=False, stop=True)
                    nc.scalar.activation(
                        hid2[cell * HMOD:(cell + 1) * HMOD,
                             cell * B:(cell + 1) * B],
                        pm, AF.Tanh, bias=m1b[:, c:c + 1],
                    )
                    wns[cell] = wnp.tile([N, B, N], F32, tag="wn",
                                         name=f"wn_{p}_{cell}")

                # ---- modulator output: delta_W^T accumulated onto W^T ----
                for k in range(NCHUNK):
                    w2t = w2p.tile([128, W2C], BF16, tag="w2t")
                    nc.gpsimd.dma_start(out=w2t,
                                        in_=w2s[p, :, k * W2C:(k + 1) * W2C])
                    dp = psd.tile([N, IBLK, 16], F32, tag="ps_d")
                    for ig in range(IBLK):
                        nc.tensor.matmul(
                            dp[:, ig, :], w2t[:, ig * N:(ig + 1) * N], hid2,
                            start=True, stop=True,
                        )
                    ks = slice(k * IBLK, (k + 1) * IBLK)
                    for cell in range(2):
                        for b in range(B):
                            nc.vector.tensor_add(
                                wns[cell][:, b, ks],
                                dp[:, :, cell * B + b],
                                wts[cell][:, b, ks],
                            )
                for cell in range(2):
                    nc.sync.dma_start(out=WnT[2 * p + cell], in_=wns[cell])

                # decay tail: o in [N*N, N*N + N)
                w2d = w2p.tile([128, N], BF16, tag="w2d")
                nc.gpsimd.dma_start(out=w2d, in_=w2s[p, :, N * N:N * N + N])
                pdd = pss.tile([N, 16], F32, tag="ps_small")
                nc.tensor.matmul(pdd, w2d, hid2, start=True, stop=True)
                dl_p = pairp.tile([N, 16], F32, tag="dl")
                nc.sync.dma_start(out=dl_p, in_=dlT[p])
                dn = pairp.tile([N, 16], F32, tag="dn")
                nc.vector.tensor_add(dn, pdd, dl_p)
                nc.sync.dma_start(out=dnT[p], in_=dn)

    nc.compile()
    return nc


_NC_CACHE = None


def _get_nc():
    global _NC_CACHE
    if _NC_CACHE is None:
        _NC_CACHE = build_bass()
    return _NC_CACHE


def _prep_inputs(h, msg, W, decay_logit, readout_drift, s_mem_live,
                 s_mem_ema_fast, H_aug_t, neuron_id, state_w1, state_b1,
                 state_w2, state_b2, msg_w1, msg_b1, msg_w2, msg_b2,
                 inject_w, inject_b, mod_w1, mod_b1, mod_w2, mod_b2):
    """Full-size host-side layout prep; returns per-core in_maps."""
    f = np.float32
    c_ = np.ascontiguousarray

    hT = c_(h.transpose(1, 3, 0, 2).astype(f))          # [NC, DN, B, N]
    msgN = c_(msg.transpose(1, 2, 0, 3).astype(f))      # [NC, N(j), B, DN]
    WT = c_(W.transpose(1, 3, 0, 2).astype(f))          # [NC, j, B, i]
    w2s = np.ascontiguousarray(mod_w2.astype(f)).reshape(NC // 2, 2 * HMOD, MOD_OUT)
    nidT_ = c_(neuron_id.transpose(0, 2, 1).astype(f))  # [NC, DN, N]
    iwT_ = c_(inject_w.transpose(0, 2, 1).astype(f))    # [NC, DN, 2*DN]
    injbT_ = c_(inject_b.T.astype(f))                   # [128, NC]
    haT_ = c_(H_aug_t.reshape(B, NC, DN).transpose(1, 2, 0).astype(f))  # [NC,DN,B]

    W_stats = np.abs(W).mean(axis=(2, 3))               # [B, NC]
    decay_mean = decay_logit.mean(axis=2)               # [B, NC]
    rd = readout_drift[:, :, 0]                         # [B, NC]
    s1 = np.broadcast_to(s_mem_live[:, None], (B, NC))
    s2 = np.broadcast_to(s_mem_ema_fast[:, None], (B, NC))
    sc5_ = c_(np.stack([W_stats, decay_mean, rd, s1, s2], axis=0)
              .transpose(2, 0, 1).astype(f))            # [NC, 5, B]

    m1w_ = mod_w1.astype(f).copy()
    m1w_[:, :2 * DN, :] *= np.float32(1.0 / N)          # fold the token mean
    m1bT_ = c_(mod_b1.T.astype(f))                      # [HMOD, NC]

    sw1T_ = c_(state_w1.T.astype(f))                    # [192, 256]
    sb1T2_ = c_(state_b1.astype(f).reshape(2, 128).T)   # [128, 2]
    sw2T_ = c_(state_w2.T.astype(f))                    # [256, 64]
    sb2T_ = c_(state_b2.astype(f).reshape(DN, 1))
    mw1T_ = c_(msg_w1.T.astype(f))                      # [128, 256]
    mb1T2_ = c_(msg_b1.astype(f).reshape(2, 128).T)
    mw2T_ = c_(msg_w2.T.astype(f))
    mb2T_ = c_(msg_b2.astype(f).reshape(DN, 1))

    # decay_logit packed [pair, n, (cell, b)]
    dlT_ = c_(decay_logit.astype(f).transpose(1, 2, 0)   # [NC, N, B]
              .reshape(NC // 2, 2, N, B).transpose(0, 2, 1, 3)
              .reshape(NC // 2, N, 16))

    in_maps = []
    for k in range(NCORES):
        cs = slice(k * NCL, (k + 1) * NCL)
        ps = slice(k * NPAIR, (k + 1) * NPAIR)
        in_maps.append({
            "hT": c_(hT[cs]), "msgN": c_(msgN[cs]), "WT": c_(WT[cs]),
            "w2s": c_(w2s[ps]), "nidT": c_(nidT_[cs]), "iwT": c_(iwT_[cs]),
            "injbT": c_(injbT_[:, cs]), "haT": c_(haT_[cs]),
            "sc5": c_(sc5_[cs]), "m1w": c_(m1w_[cs]),
            "m1bT": c_(m1bT_[:, cs]), "sw1T": sw1T_, "sb1T2": sb1T2_,
            "sw2T": sw2T_, "sb2T": sb2T_, "mw1T": mw1T_, "mb1T2": mb1T2_,
            "mw2T": mw2T_, "mb2T": mb2T_, "dlT": c_(dlT_[ps]),
        })
    return in_maps


def _post_outputs(results, mod_b2):
    f = np.float32
    hnT = np.concatenate([r["hnT"] for r in results], axis=0)   # [nc, DN, B, N]
    mnT = np.concatenate([r["mnT"] for r in results], axis=0)
    WnT = np.concatenate([r["WnT"] for r in results], axis=0)   # [nc, j, B, i]
    dnT = np.concatenate([r["dnT"] for r in results], axis=0)   # [nc/2, N, 16]
    ncells = hnT.shape[0]
    mod_b2 = mod_b2[:ncells]

    h_new = hnT.transpose(2, 0, 3, 1)                           # [B, nc, N, DN]
    msg_new = mnT.transpose(2, 0, 3, 1)
    W_new = (WnT.transpose(2, 0, 3, 1)
             + mod_b2[:, :N * N].reshape(ncells, N, N)[None].astype(f))
    decay_new = (dnT.reshape(ncells // 2, N, 2, B).transpose(3, 0, 2, 1)
                 .reshape(B, ncells, N)
                 + mod_b2[:, N * N:].reshape(ncells, N)[None].astype(f))
    return (np.ascontiguousarray(h_new.astype(f)),
            np.ascontiguousarray(msg_new.astype(f)),
            np.ascontiguousarray(W_new.astype(f)),
            np.ascontiguousarray(decay_new.astype(f)))


def kernel(**inputs):
    nc = _get_nc()
    in_maps = _prep_inputs(**{k: np.asarray(v) for k, v in inputs.items()})
    res = run_bass_kernel_spmd(nc, in_maps, core_ids=list(range(NCORES)))
    return _post_outputs(res.results, np.asarray(inputs["mod_b2"]))


if __name__ == "__main__":
    import reference
    ins = {k: np.asarray(v) for k, v in reference.setup_inputs().items()}
    outs = kernel(**ins)
    exp = reference.reference(**ins)
    for name, a, e in zip(("h_new", "msg_new", "W_new", "decay_new"), outs, exp):
        e = np.asarray(e)
        err = np.abs(a - e).max() / max(np.abs(e).max(), 1e-30)
        print(f"{name}: rel-err {err:.3e}")


# revision 14
# speedup vs baseline: 1.0812x; 1.0812x over previous
"""Trainium2 Bass kernel for nn_MemoryGraph (gnn_message_passing).

Sharding: N_cells=64 split 8 ways -> 8 cells per NeuronCore. Each cell is
fully independent (per-cell weights + per-cell state), so there are no
collectives. Batch B=8 stays whole on every core.

Device-side layout is feature-major ([feat, token]) everywhere so that every
matmul contracts over the partition dim with zero on-device transposes; all
transposes are host-side numpy work:
  in : h^T, W^T, neuron_id^T, inject_w^T, H_aug^T, decay_logit (pair-packed)
  out: h_new^T, msg_new^T, (W+delta_W)^T, decay-delta pair-packed
The modulator's big matmul streams mod_w2 for a *pair* of cells as a
[128, 16512] stationary block (two 64-row hh blocks stacked -- contiguous in
DRAM) against a zero-padded [128, 16] hidden block, giving delta^T directly
partition-dense in PSUM. mod_b2 is added on host afterwards.
"""

import sys

for _p in ("/opt/trn_rl_repo",):
    if _p not in sys.path:
        sys.path.insert(0, _p)

import numpy as np

import concourse.bass as bass
import concourse.tile as tile
from concourse import bacc, mybir
from concourse.bass import _add_dep_helper
from concourse.bass_utils import run_bass_kernel_spmd


def _dep(from_inst, to_inst, reason):
    _add_dep_helper(getattr(from_inst, "ins", from_inst),
                    getattr(to_inst, "ins", to_inst), reason=reason)

F32 = mybir.dt.float32
BF16 = mybir.dt.bfloat16
F32R = mybir.dt.float32r
AX = mybir.AxisListType
AF = mybir.ActivationFunctionType

B, NC, N, DN, ALPHA = 8, 64, 128, 64, 2
HS, HM, HMOD = 256, 256, 64
MOD_IN, MOD_OUT = 2 * DN + 5, N * N + N
NCORES = 8
NCL = NC // NCORES          # cells per core = 8
NPAIR = NCL // 2            # cell pairs per core = 4
W2C = 4096                  # mod_w2 free-dim chunk (32 i-blocks)
NCHUNK = (N * N) // W2C     # 4 chunks for the delta_W part
IBLK = W2C // N             # i-blocks per chunk = 32


def build_bass():
    nc = bacc.Bacc("TRN2", target_bir_lowering=False, debug=False)

    def din(name, shape):
        return nc.dram_tensor(name, list(shape), F32, kind="ExternalInput").ap()

    def dout(name, shape):
        return nc.dram_tensor(name, list(shape), F32, kind="ExternalOutput").ap()

    hT = din("hT", (NCL, DN, B, N))            # h^T packed per cell
    msgN = din("msgN", (NCL, N, B, DN))        # msg [j, (b, d)] packed per cell
    WT = din("WT", (NCL, N, B, N))             # W^T packed per cell: [j, (b, i)]
    w2s = din("w2s", (NPAIR, 2 * HMOD, MOD_OUT))  # stacked mod_w2 per pair
    nidT = din("nidT", (NCL, DN, N))
    iwT = din("iwT", (NCL, DN, ALPHA * DN))
    injbT = din("injbT", (ALPHA * DN, NCL))
    haT = din("haT", (NCL, DN, B))
    sc5 = din("sc5", (NCL, 5, B))
    m1w = din("m1w", (NCL, MOD_IN, HMOD))      # mod_w1 (rows 0:128 prescaled by 1/N)
    m1bT = din("m1bT", (HMOD, NCL))
    sw1T = din("sw1T", (3 * DN, HS))
    sb1T2 = din("sb1T2", (128, 2))
    sw2T = din("sw2T", (HS, DN))
    sb2T = din("sb2T", (DN, 1))
    mw1T = din("mw1T", (2 * DN, HM))
    mb1T2 = din("mb1T2", (128, 2))
    mw2T = din("mw2T", (HM, DN))
    mb2T = din("mb2T", (DN, 1))
    dlT = din("dlT", (NPAIR, N, 16))           # decay_logit [pair, n, (cell,b)]

    hnT = dout("hnT", (NCL, DN, B, N))
    mnT = dout("mnT", (NCL, DN, B, N))
    WnT = dout("WnT", (NCL, N, B, N))          # (W + delta_W)^T, [j, (b, i)]
    dnT = dout("dnT", (NPAIR, N, 16))          # decay_logit + delta_decay

    def r(ap):
        return ap.bitcast(F32R)

    with tile.TileContext(nc) as tc:
        with (
            tc.tile_pool(name="const", bufs=1) as constp,
            tc.tile_pool(name="cellc", bufs=2) as cellp,
            tc.tile_pool(name="pairc", bufs=2) as pairp,
            tc.tile_pool(name="wt", bufs=4) as wtp,
            tc.tile_pool(name="wn", bufs=4) as wnp,
            tc.tile_pool(name="w2", bufs=4) as w2p,
            tc.tile_pool(name="acts", bufs=4) as actp,
            tc.tile_pool(name="outs", bufs=3) as outp,
            tc.tile_pool(name="ps_l1", bufs=2, space="PSUM") as psl1,
            tc.tile_pool(name="ps_s", bufs=3, space="PSUM") as pss,
            tc.tile_pool(name="ps_d", bufs=2, space="PSUM") as psd,
        ):
            # ---- resident shared weights (round-copied to f32r for PE) ----
            stg = constp.tile([128, 2 * HS], F32, tag="stg")
            sw1a = constp.tile([128, HS], F32, tag="sw1a")   # state_w1^T rows 0:128
            sw1b = constp.tile([64, HS], F32, tag="sw1b")    # rows 128:192 (nid part)
            nc.sync.dma_start(out=stg[:, 0:HS], in_=sw1T[0:128, :])
            nc.sync.dma_start(out=stg[0:64, HS:2 * HS], in_=sw1T[128:192, :])
            nc.vector.tensor_copy(sw1a.bitcast(F32R), stg[:, 0:HS])
            nc.vector.tensor_copy(sw1b.bitcast(F32R), stg[0:64, HS:2 * HS])
            sb1 = constp.tile([128, 2], F32, tag="sb1")
            nc.sync.dma_start(out=sb1, in_=sb1T2)
            stg2 = constp.tile([128, 2, DN], F32, tag="stg2")
            sw2k = constp.tile([128, 2, DN], F32, tag="sw2k")  # state_w2^T k-chunks
            nc.sync.dma_start(out=stg2[:, 0, :], in_=sw2T[0:128, :])
            nc.sync.dma_start(out=stg2[:, 1, :], in_=sw2T[128:256, :])
            nc.vector.tensor_copy(sw2k.bitcast(F32R), stg2)
            sb2 = constp.tile([DN, 1], F32, tag="sb2")
            nc.sync.dma_start(out=sb2, in_=sb2T)
            stg3 = constp.tile([128, HM], F32, tag="stg3")
            mw1 = constp.tile([128, HM], F32, tag="mw1")
            nc.sync.dma_start(out=stg3, in_=mw1T)
            nc.vector.tensor_copy(mw1.bitcast(F32R), stg3)
            mb1 = constp.tile([128, 2], F32, tag="mb1")
            nc.sync.dma_start(out=mb1, in_=mb1T2)
            stg4 = constp.tile([128, 2, DN], F32, tag="stg4")
            mw2k = constp.tile([128, 2, DN], F32, tag="mw2k")
            nc.sync.dma_start(out=stg4[:, 0, :], in_=mw2T[0:128, :])
            nc.sync.dma_start(out=stg4[:, 1, :], in_=mw2T[128:256, :])
            nc.vector.tensor_copy(mw2k.bitcast(F32R), stg4)
            mb2 = constp.tile([DN, 1], F32, tag="mb2")
            nc.sync.dma_start(out=mb2, in_=mb2T)
            injb = constp.tile([ALPHA * DN, NCL], F32, tag="injb")
            nc.sync.dma_start(out=injb, in_=injbT)
            m1b = constp.tile([HMOD, NCL], F32, tag="m1b")
            nc.sync.dma_start(out=m1b, in_=m1bT)

            # ~4us of dummy matmuls to take the PE HAM clock-gate to 8/8
            # before the first real matmuls arrive.
            ps_warm = pss.tile([128, HS], F32, tag="ps_warm", bufs=1)
            for _ in range(40):
                nc.tensor.matmul(ps_warm, r(sw1a[:, 0:128]), r(sw1a),
                                 start=True, stop=True)

            gate_insts = [None, None]
            for p in range(NPAIR):
                hid2 = pairp.tile([128, 16], BF16, tag="hid2")
                nc.vector.memset(hid2, 0.0)
                wts = [None, None]
                wns = [None, None]

                for cell in range(2):
                    c = 2 * p + cell
                    nid_c = cellp.tile([DN, N], F32, tag="nid")
                    nc.sync.dma_start(out=nid_c, in_=nidT[c])
                    nid4 = cellp.tile([DN, 4, N], F32, tag="nid4")
                    for bl in range(4):
                        nc.vector.tensor_copy(nid4[:, bl, :].bitcast(F32R), nid_c)
                    iw_c = cellp.tile([DN, ALPHA * DN], F32, tag="iw")
                    nc.sync.dma_start(out=iw_c, in_=iwT[c])
                    ha_c = cellp.tile([DN, B], F32, tag="ha")
                    nc.sync.dma_start(out=ha_c, in_=haT[c])
                    m1wa = cellp.tile([128, HMOD], F32, tag="m1wa")
                    nc.sync.dma_start(out=m1wa, in_=m1w[c, 0:128, :])
                    m1wb = cellp.tile([5, HMOD], F32, tag="m1wb")
                    nc.sync.dma_start(out=m1wb, in_=m1w[c, 128:133, :])
                    mi_a = cellp.tile([128, B], F32, tag="mi_a")
                    mi_b = cellp.tile([5, B], F32, tag="mi_b")
                    nc.sync.dma_start(out=mi_b, in_=sc5[c])

                    # inject = (H_aug cell slice) @ inject_w^T, per ALPHA row
                    pi = pss.tile([DN, ALPHA, B], F32, tag="ps_small")
                    for a in range(ALPHA):
                        nc.tensor.matmul(
                            pi[:, a, :], iw_c[:, a * DN:(a + 1) * DN], ha_c,
                            start=True, stop=True,
                        )
                    inj = cellp.tile([DN, ALPHA, B], F32, tag="inj")
                    for a in range(ALPHA):
                        nc.scalar.activation(
                            inj[:, a, :], pi[:, a, :], AF.Identity,
                            bias=injb[a * DN:(a + 1) * DN, c:c + 1],
                        )

                    # per-cell bulk loads (big DMA descriptors)
                    wt_c = wtp.tile([N, B, N], F32, tag="wt",
                                    name=f"wt_{p}_{cell}")
                    wts[cell] = wt_c
                    wt_dma = nc.sync.dma_start(out=wt_c, in_=WT[c])
                    if p == 0:
                        gate_insts[cell] = wt_dma
                    msg_c = cellp.tile([N, B, DN], F32, tag="msgc")
                    nc.sync.dma_start(out=msg_c, in_=msgN[c])

                    st8 = actp.tile([128, B, N], F32, tag="st")
                    mg8 = actp.tile([128, B, N], F32, tag="mg")
                    mn8 = outp.tile([DN, B, N], F32, tag="mn")
                    hstg = cellp.tile([DN, B, N], F32, tag="hstg")
                    nc.sync.dma_start(out=hstg, in_=hT[c])
                    nc.scalar.copy(st8[0:DN].bitcast(F32R), hstg)

                    for b in range(B):
                        # received^T = (msg^T) @ (W^T) : [d, i]
                        pr = pss.tile([DN, N], F32, tag="ps_small")
                        nc.tensor.matmul(pr, msg_c[:, b, :], wt_c[:, b, :],
                                         start=True, stop=True)
                        nc.vector.tensor_add(
                            pr[:, 0:ALPHA], pr[:, 0:ALPHA], inj[:, :, b]
                        )
                        nc.vector.tensor_copy(st8[DN:128, b, :].bitcast(F32R),
                                              pr)
                    for g in range(2):
                        nc.vector.tensor_copy(
                            mg8[DN:128, 4 * g:4 * g + 4, :].bitcast(F32R),
                            st8[DN:128, 4 * g:4 * g + 4, :])

                    for g in range(2):          # token batches of 4*N
                        g4 = slice(4 * g, 4 * g + 4)
                        # state MLP layer 1
                        hid1 = actp.tile([128, 2, 4 * N], F32, tag="hid1")
                        for m in range(2):
                            ps1 = psl1.tile([128, 4 * N], F32, tag="ps_l1")
                            nc.tensor.matmul(
                                ps1, r(sw1a[:, m * 128:(m + 1) * 128]),
                                r(st8[:, g4, :]), start=True, stop=False,
                            )
                            nc.tensor.matmul(
                                ps1, r(sw1b[:, m * 128:(m + 1) * 128]), r(nid4),
                                start=False, stop=True,
                            )
                            nc.scalar.activation(
                                hid1[:, m, :].bitcast(F32R), ps1, AF.Tanh,
                                bias=sb1[:, m:m + 1]
                            )
                        # state MLP layer 2 -> h_new^T into mg8 rows 0:64
                        ps2 = pss.tile([DN, 4 * N], F32, tag="ps_small")
                        nc.tensor.matmul(ps2, r(sw2k[:, 0, :]),
                                         r(hid1[:, 0, :]), start=True, stop=False)
                        nc.tensor.matmul(ps2, r(sw2k[:, 1, :]),
                                         r(hid1[:, 1, :]), start=False, stop=True)
                        mg8f = mg8[0:DN, g4, :].rearrange("d b n -> d (b n)")
                        nc.scalar.activation(mg8f.bitcast(F32R), ps2, AF.Tanh,
                                             bias=sb2)
                        for bl in range(4):
                            b = g * 4 + bl
                            nc.vector.reduce_sum(mi_a[0:DN, b:b + 1],
                                                 mg8[0:DN, b, :], axis=AX.X)

                        # msg MLP
                        mid1 = actp.tile([128, 2, 4 * N], F32, tag="mid1")
                        for m in range(2):
                            ps1 = psl1.tile([128, 4 * N], F32, tag="ps_l1")
                            nc.tensor.matmul(
                                ps1, r(mw1[:, m * 128:(m + 1) * 128]),
                                r(mg8[:, g4, :]), start=True, stop=True,
                            )
                            nc.scalar.activation(
                                mid1[:, m, :].bitcast(F32R), ps1, AF.Tanh,
                                bias=mb1[:, m:m + 1]
                            )
                        ps3 = pss.tile([DN, 4 * N], F32, tag="ps_small")
                        nc.tensor.matmul(ps3, r(mw2k[:, 0, :]),
                                         r(mid1[:, 0, :]), start=True, stop=False)
                        nc.tensor.matmul(ps3, r(mw2k[:, 1, :]),
                                         r(mid1[:, 1, :]), start=False, stop=True)
                        mn8f = mn8[:, g4, :].rearrange("d b n -> d (b n)")
                        nc.scalar.activation(mn8f, ps3, AF.Tanh, bias=mb2)
                        for bl in range(4):
                            b = g * 4 + bl
                            nc.vector.reduce_sum(mi_a[DN:128, b:b + 1],
                                                 mn8[:, b, :], axis=AX.X)

                    nc.sync.dma_start(out=hnT[c], in_=mg8[0:DN])
                    nc.sync.dma_start(out=mnT[c], in_=mn8)

                    # modulator hidden for this cell -> hid2 block (bf16)
                    pm = pss.tile([HMOD, B], F32, tag="ps_small")
                    nc.tensor.matmul(pm, m1wa, mi_a, start=True, stop=False)
                    nc.tensor.matmul(pm, m1wb, mi_b, start=False, stop=True)
                    nc.scalar.activation(
                        hid2[cell * HMOD:(cell + 1) * HMOD,
                             cell * B:(cell + 1) * B],
                        pm, AF.Tanh, bias=m1b[:, c:c + 1],
                    )
                    wns[cell] = wnp.tile([N, B, N], F32, tag="wn",
                                         name=f"wn_{p}_{cell}")

                # ---- modulator output: delta_W^T accumulated onto W^T ----
                for k in range(NCHUNK):
                    w2t = w2p.tile([128, W2C], BF16, tag="w2t")
                    w2_dma = nc.gpsimd.dma_start(
                        out=w2t, in_=w2s[p, :, k * W2C:(k + 1) * W2C])
                    if p == 0:
                        # keep pair-0 w2 prefetch out of the DMA rings until
                        # the first cells' working-set loads are in flight
                        _dep(w2_dma, gate_insts[min(k // 2, 1)],
                             "w2 prefetch after first-cell loads")
                    dp = psd.tile([N, IBLK, 16], F32, tag="ps_d")
                    for ig in range(IBLK):
                        nc.tensor.matmul(
                            dp[:, ig, :], w2t[:, ig * N:(ig + 1) * N], hid2,
                            start=True, stop=True,
                        )
                    ks = slice(k * IBLK, (k + 1) * IBLK)
                    for cell in range(2):
                        for b in range(B):
                            nc.vector.tensor_add(
                                wns[cell][:, b, ks],
                                dp[:, :, cell * B + b],
                                wts[cell][:, b, ks],
                            )
                for cell in range(2):
                    nc.sync.dma_start(out=WnT[2 * p + cell], in_=wns[cell])

                # decay tail: o in [N*N, N*N + N)
                w2d = w2p.tile([128, N], BF16, tag="w2d")
                nc.gpsimd.dma_start(out=w2d, in_=w2s[p, :, N * N:N * N + N])
                pdd = pss.tile([N, 16], F32, tag="ps_small")
                nc.tensor.matmul(pdd, w2d, hid2, start=True, stop=True)
                dl_p = pairp.tile([N, 16], F32, tag="dl")
                nc.sync.dma_start(out=dl_p, in_=dlT[p])
                dn = pairp.tile([N, 16], F32, tag="dn")
                nc.vector.tensor_add(dn, pdd, dl_p)
                nc.sync.dma_start(out=dnT[p], in_=dn)

    nc.compile()
    return nc


_NC_CACHE = None


def _get_nc():
    global _NC_CACHE
    if _NC_CACHE is None:
        _NC_CACHE = build_bass()
    return _NC_CACHE


def _prep_inputs(h, msg, W, decay_logit, readout_drift, s_mem_live,
                 s_mem_ema_fast, H_aug_t, neuron_id, state_w1, state_b1,
                 state_w2, state_b2, msg_w1, msg_b1, msg_w2, msg_b2,
                 inject_w, inject_b, mod_w1, mod_b1, mod_w2, mod_b2):
    """Full-size host-side layout prep; returns per-core in_maps."""
    f = np.float32
    c_ = np.ascontiguousarray

    hT = c_(h.transpose(1, 3, 0, 2).astype(f))          # [NC, DN, B, N]
    msgN = c_(msg.transpose(1, 2, 0, 3).astype(f))      # [NC, N(j), B, DN]
    WT = c_(W.transpose(1, 3, 0, 2).astype(f))          # [NC, j, B, i]
    w2s = np.ascontiguousarray(mod_w2.astype(f)).reshape(NC // 2, 2 * HMOD, MOD_OUT)
    nidT_ = c_(neuron_id.transpose(0, 2, 1).astype(f))  # [NC, DN, N]
    iwT_ = c_(inject_w.transpose(0, 2, 1).astype(f))    # [NC, DN, 2*DN]
    injbT_ = c_(inject_b.T.astype(f))                   # [128, NC]
    haT_ = c_(H_aug_t.reshape(B, NC, DN).transpose(1, 2, 0).astype(f))  # [NC,DN,B]

    W_stats = np.abs(W).mean(axis=(2, 3))               # [B, NC]
    decay_mean = decay_logit.mean(axis=2)               # [B, NC]
    rd = readout_drift[:, :, 0]                         # [B, NC]
    s1 = np.broadcast_to(s_mem_live[:, None], (B, NC))
    s2 = np.broadcast_to(s_mem_ema_fast[:, None], (B, NC))
    sc5_ = c_(np.stack([W_stats, decay_mean, rd, s1, s2], axis=0)
              .transpose(2, 0, 1).astype(f))            # [NC, 5, B]

    m1w_ = mod_w1.astype(f).copy()
    m1w_[:, :2 * DN, :] *= np.float32(1.0 / N)          # fold the token mean
    m1bT_ = c_(mod_b1.T.astype(f))                      # [HMOD, NC]

    sw1T_ = c_(state_w1.T.astype(f))                    # [192, 256]
    sb1T2_ = c_(state_b1.astype(f).reshape(2, 128).T)   # [128, 2]
    sw2T_ = c_(state_w2.T.astype(f))                    # [256, 64]
    sb2T_ = c_(state_b2.astype(f).reshape(DN, 1))
    mw1T_ = c_(msg_w1.T.astype(f))                      # [128, 256]
    mb1T2_ = c_(msg_b1.astype(f).reshape(2, 128).T)
    mw2T_ = c_(msg_w2.T.astype(f))
    mb2T_ = c_(msg_b2.astype(f).reshape(DN, 1))

    # decay_logit packed [pair, n, (cell, b)]
    dlT_ = c_(decay_logit.astype(f).transpose(1, 2, 0)   # [NC, N, B]
              .reshape(NC // 2, 2, N, B).transpose(0, 2, 1, 3)
              .reshape(NC // 2, N, 16))

    in_maps = []
    for k in range(NCORES):
        cs = slice(k * NCL, (k + 1) * NCL)
        ps = slice(k * NPAIR, (k + 1) * NPAIR)
        in_maps.append({
            "hT": c_(hT[cs]), "msgN": c_(msgN[cs]), "WT": c_(WT[cs]),
            "w2s": c_(w2s[ps]), "nidT": c_(nidT_[cs]), "iwT": c_(iwT_[cs]),
            "injbT": c_(injbT_[:, cs]), "haT": c_(haT_[cs]),
            "sc5": c_(sc5_[cs]), "m1w": c_(m1w_[cs]),
            "m1bT": c_(m1bT_[:, cs]), "sw1T": sw1T_, "sb1T2": sb1T2_,
            "sw2T": sw2T_, "sb2T": sb2T_, "mw1T": mw1T_, "mb1T2": mb1T2_,
            "mw2T": mw2T_, "mb2T": mb2T_, "dlT": c_(dlT_[ps]),
        })
    return in_maps


def _post_outputs(results, mod_b2):
    f = np.float32
    hnT = np.concatenate([r["hnT"] for r in results], axis=0)   # [nc, DN, B, N]
    mnT = np.concatenate([r["mnT"] for r in results], axis=0)
    WnT = np.concatenate([r["WnT"] for r in results], axis=0)   # [nc, j, B, i]
    dnT = np.concatenate([r["dnT"] for r in results], axis=0)   # [nc/2, N, 16]
    ncells = hnT.shape[0]
    mod_b2 = mod_b2[:ncells]

    h_new = hnT.transpose(2, 0, 3, 1)                           # [B, nc, N, DN]
    msg_new = mnT.transpose(2, 0, 3, 1)
    W_new = (WnT.transpose(2, 0, 3, 1)
             + mod_b2[:, :N * N].reshape(ncells, N, N)[None].astype(f))
    decay_new = (dnT.reshape(ncells // 2, N, 2, B).transpose(3, 0, 2, 1)
                 .reshape(B, ncells, N)
                 + mod_b2[:, N * N:].reshape(ncells, N)[None].astype(f))
    return (np.ascontiguousarray(h_new.astype(f)),
            np.ascontiguousarray(msg_new.astype(f)),
            np.ascontiguousarray(W_new.astype(f)),
            np.ascontiguousarray(decay_new.astype(f)))


def kernel(**inputs):
    nc = _get_nc()
    in_maps = _prep_inputs(**{k: np.asarray(v) for k, v in inputs.items()})
    res = run_bass_kernel_spmd(nc, in_maps, core_ids=list(range(NCORES)))
    return _post_outputs(res.results, np.asarray(inputs["mod_b2"]))


if __name__ == "__main__":
    import reference
    ins = {k: np.asarray(v) for k, v in reference.setup_inputs().items()}
    outs = kernel(**ins)
    exp = reference.reference(**ins)
    for name, a, e in zip(("h_new", "msg_new", "W_new", "decay_new"), outs, exp):
        e = np.asarray(e)
        err = np.abs(a - e).max() / max(np.abs(e).max(), 1e-30)
        print(f"{name}: rel-err {err:.3e}")
